# revision 22
# baseline (speedup 1.0000x reference)
"""Depthwise Conv1d (C=512, K=3, stride=1, pad=1) on 8 Trainium2 NeuronCores.

Problem: x [16, 512, 4096] f32, w [512, 1, 3] f32, b [512] f32
         out[n,c,l] = sum_k w[c,0,k] * x_pad[n,c,l+k] + b[c]

Sharding: data-parallel over batch — 2 batches per core; each core handles
all 512 channels as 4 blocks of 128 partitions -> 8 rows of [128, 4096].

Computes in bf16 (host quantizes inputs / dequantizes the output):
rel-err ~3e-3 << the 2e-2 gate.  The host also bakes the two zero pad
columns into the uploaded tensor, so a row load is one contiguous
[128, 4098] transfer and no on-device memsets are needed.

Work is split across three engines (HW-measured op costs, including
the ~20% slowdown all engines see while DMA traffic is in flight):

PE rows (0, 2, 3, 4) and DVE+ScalarE rows (1, 5, 6, 7):
  ScalarE  tC = Identity(x1*w1 + b)          ~3.8us  (1 elem/cyc)
  VectorE  tA = ts(x0*w0)   4x mode  ~1.27us
           tB = ts(x2*w2)   4x mode  ~1.27us  (row 5: on ScalarE)
           s  = tt(tA+tC)   2x mode  ~2.28us
           ot = tt(s+tB)    2x mode  ~2.28us  (adds in-place into tA)
  (scalar_tensor_tensor has NO fast DVE uop -> 1x; avoided entirely.
   GpSimd tensor_tensor works but contends for the DVE SBUF port and
   slows concurrent DVE ops 4-7x -> not used.)

TensorE rows (2, 3, 4), back-to-back so the PE stays at full p-state:
  per half-row [128,2048]: 4 col-chunks x 3 taps of diag(w_k) matmuls
  accumulating in PSUM f32 (moving free dim capped at 512), then
  ScalarE evacuates: ot = Identity(psum + b) -> SBUF bf16 (~1.85us).

DMA: loads + stores interleaved on the sync-engine HWDGE ring in
pipeline-flow order; weights on the scalar-engine ring.  Rows 0 and 7
run in half-row chunks to shorten the pipeline head and tail.
"""

import numpy as np

B, C, L, K = 16, 512, 4096, 3
N_CORES = 8
B_SH = B // N_CORES          # 2 batches per core
NBLK = C // 128              # 4 channel blocks
NROW = B_SH * NBLK           # 8 rows of [128, 4096] per core
HALF = L // 2
LP = L + 2                   # padded row length
MM = 512                     # max moving free dim per matmul

_STATE = {}


def _build_program():
    from contextlib import ExitStack

    import concourse.bacc as bacc
    import concourse.mybir as mybir
    import concourse.tile as tile

    f32 = mybir.dt.float32
    bf16 = mybir.dt.bfloat16
    MULT = mybir.AluOpType.mult
    ADD = mybir.AluOpType.add
    IDENT = mybir.ActivationFunctionType.Identity

    nc = bacc.Bacc(
        "TRN2",
        target_bir_lowering=False,
        debug=False,
        num_devices=N_CORES,
    )
    x_d = nc.dram_tensor("x", [B_SH, C, LP], bf16, kind="ExternalInput").ap()
    wp_d = nc.dram_tensor("wpack", [128, 4 * NBLK], f32, kind="ExternalInput").ap()
    wd_d = nc.dram_tensor(
        "wdiag", [128, 3 * NBLK * 128], bf16, kind="ExternalInput"
    ).ap()
    o_d = nc.dram_tensor("out", [B_SH, C, L], bf16, kind="ExternalOutput").ap()

    x3 = x_d.rearrange("b (k p) l -> (b k) p l", p=128)
    o3 = o_d.rearrange("b (k p) l -> (b k) p l", p=128)

    with tile.TileContext(nc) as tc, ExitStack() as ctx:
        wpool = ctx.enter_context(tc.tile_pool(name="wpool", bufs=1))
        xpool = ctx.enter_context(tc.tile_pool(name="xpool", bufs=7))
        apool = ctx.enter_context(tc.tile_pool(name="apool", bufs=3))
        bpool = ctx.enter_context(tc.tile_pool(name="bpool", bufs=3))
        cpool = ctx.enter_context(tc.tile_pool(name="cpool", bufs=3))
        hpool = ctx.enter_context(tc.tile_pool(name="hpool", bufs=2))
        epool = ctx.enter_context(tc.tile_pool(name="epool", bufs=4))
        pspool = ctx.enter_context(tc.tile_pool(name="pspool", bufs=2, space="PSUM"))

        wdiag = wpool.tile([128, 3 * NBLK * 128], bf16)
        nc.scalar.dma_start(wdiag[:, :], wd_d)
        wtile = wpool.tile([128, 4 * NBLK], f32)
        nc.scalar.dma_start(wtile[:, :], wp_d)

        def wsl(r, j):  # w0/w1/w2/bias column for row r's channel block
            blk = r % NBLK
            return wtile[:, blk * 4 + j : blk * 4 + j + 1]

        def wdg(r, k):  # [128,128] diag(w_k) for row r's channel block
            blk = r % NBLK
            o = (blk * 3 + k) * 128
            return wdiag[:, o : o + 128]

        xps = {}

        QB = [0, 1026, 2050, 3074, LP]  # quarter boundaries (incl. halo)

        def load(r, split=1):
            xps[r] = xpool.tile([128, LP], bf16, tag="xp", name=f"xp{r}")
            if split == 1:
                nc.sync.dma_start(xps[r][:, :], x3[r])
            elif split == -2:
                nc.scalar.dma_start(xps[r][:, 0:2050], x3[r][:, 0:2050])
                nc.scalar.dma_start(xps[r][:, 2050:LP], x3[r][:, 2050:LP])
            elif split == 2:
                nc.sync.dma_start(xps[r][:, 0:2050], x3[r][:, 0:2050])
                nc.sync.dma_start(xps[r][:, 2050:LP], x3[r][:, 2050:LP])
            else:
                for q in range(4):
                    a, b = QB[q], QB[q + 1]
                    nc.sync.dma_start(xps[r][:, a:b], x3[r][:, a:b])

        tA = {}
        tB = {}
        tC = {}
        ots = {}

        def actC(r, lo, n, tag, name):
            t = cpool.tile(
                [128, n], bf16, tag=tag, name=name,
                bufs=2 if tag == "tCh" else None,
            )
            nc.scalar.activation(
                t[:, :], xps[r][:, lo + 1 : lo + 1 + n],
                IDENT, bias=wsl(r, 3), scale=wsl(r, 1),
            )
            return t

        def actB(r):
            t = bpool.tile([128, L], bf16, tag="tB", name=f"tB{r}")
            nc.scalar.activation(
                t[:, :], xps[r][:, 2:LP], IDENT, scale=wsl(r, 2),
            )
            return t

        def actC_half(r, t, h):
            lo = h * HALF
            nc.scalar.activation(
                t[:, lo : lo + HALF], xps[r][:, lo + 1 : lo + 1 + HALF],
                IDENT, bias=wsl(r, 3), scale=wsl(r, 1),
            )

        def actB_half(r, t, h):
            lo = h * HALF
            nc.scalar.activation(
                t[:, lo : lo + HALF], xps[r][:, lo + 2 : lo + 2 + HALF],
                IDENT, scale=wsl(r, 2),
            )

        def ts_full(r, j, pool, tag, name):
            t = pool.tile([128, L], bf16, tag=tag, name=name)
            nc.vector.tensor_scalar(
                t[:, :], xps[r][:, j : j + L], wsl(r, j), None, MULT
            )
            return t

        def row_half(r, h):
            """full chain for one half of row r on DVE; returns ot tile"""
            lo = h * HALF
            xp = xps[r]
            ta = hpool.tile([128, HALF], bf16, tag="tAh", name=f"tA{r}{h}")
            nc.vector.tensor_scalar(
                ta[:, :], xp[:, lo : lo + HALF], wsl(r, 0), None, MULT
            )
            tb = hpool.tile([128, HALF], bf16, tag="tBh", name=f"tB{r}{h}")
            nc.vector.tensor_scalar(
                tb[:, :], xp[:, lo + 2 : lo + 2 + HALF], wsl(r, 2), None, MULT
            )
            nc.vector.tensor_tensor(ta[:, :], ta[:, :], tC[(r, h)][:, :], ADD)
            nc.vector.tensor_tensor(ta[:, :], ta[:, :], tb[:, :], ADD)
            return ta

        def dve_add(dst, other):
            nc.vector.tensor_tensor(dst[:, :], dst[:, :], other[:, :], ADD)

        def pe_h(r, h):
            """12 accumulating diag matmuls for one half of row r -> psum"""
            lo = h * HALF
            ps = pspool.tile([128, HALF], f32, tag="ps", name=f"ps{r}{h}")
            for c in range(HALF // MM):
                for k in range(3):
                    nc.tensor.matmul(
                        ps[:, c * MM : (c + 1) * MM],
                        wdg(r, k),
                        xps[r][:, lo + k + c * MM : lo + k + c * MM + MM],
                        start=(k == 0),
                        stop=(k == 2),
                    )
            return ps

        def evac_h(r, h, ps):
            t = epool.tile([128, HALF], bf16, tag="ev", name=f"ev{r}{h}")
            nc.scalar.activation(
                t[:, :], ps[:, :], IDENT, bias=wsl(r, 3), scale=1.0,
            )
            ots[(r, h)] = t
            return t

        def store(r, h=None):
            if h is None:
                nc.sync.dma_start(o3[r], ots[r][:, :])
            else:
                lo = h * HALF
                nc.sync.dma_start(o3[r][:, lo : lo + HALF], ots[(r, h)][:, :])

        def row_full_dve(r):
            """all-DVE full row: center tap via 2-scalar ts (w1*x + b)"""
            ta = ts_full(r, 0, apool, "tA", f"tA{r}")
            tb = ts_full(r, 2, bpool, "tB", f"tB{r}")
            tc = cpool.tile([128, L], bf16, tag="tC", name=f"tC{r}")
            nc.vector.tensor_scalar(
                tc[:, :], xps[r][:, 1 : 1 + L], wsl(r, 1), wsl(r, 3), MULT, ADD
            )
            dve_add(ta, tc)
            dve_add(ta, tb)
            ots[r] = ta
            return ta

        def row_half_dve2(r, ta, h):
            """all-DVE half chain into full-row tile ta"""
            lo = h * HALF
            xp = xps[r]
            tb = hpool.tile([128, HALF], bf16, tag="tBh", name=f"tBh{r}{h}")
            nc.vector.tensor_scalar(
                tb[:, :], xp[:, lo + 2 : lo + 2 + HALF], wsl(r, 2), None, MULT
            )
            tc = hpool.tile([128, HALF], bf16, tag="tCh2", name=f"tCd{r}{h}")
            nc.vector.tensor_scalar(
                tc[:, :], xp[:, lo + 1 : lo + 1 + HALF], wsl(r, 1), wsl(r, 3),
                MULT, ADD,
            )
            sl = ta[:, lo : lo + HALF]
            nc.vector.tensor_scalar(
                sl, xp[:, lo : lo + HALF], wsl(r, 0), None, MULT
            )
            nc.vector.tensor_tensor(sl, sl, tc[:, :], ADD)
            nc.vector.tensor_tensor(sl, sl, tb[:, :], ADD)

        # ---- emission in pipeline-flow order (per-queue program order ----
        # ---- is the schedule; DMA-completion sems are batched per ring) ----
        # loads: 0,1,2,7,5,3,4,6; ld1 rides the scalar ring in halves so
        # DVE work starts early without waiting on the sync ring.
        load(0, split=4)
        load(1, split=-2)                  # scalar ring, halves
        load(2)
        tA[1] = apool.tile([128, L], bf16, tag="tA", name="tA1")
        row_half_dve2(1, tA[1], 0)
        ots[1] = tA[1]
        load(7)
        ps = pe_h(0, 0)
        evac_h(0, 0, ps)
        row_half_dve2(1, tA[1], 1)
        load(5)
        tC[5] = cpool.tile([128, L], bf16, tag="tC", name="tC5")
        actC_half(5, tC[5], 0)
        ps = pe_h(0, 1)
        evac_h(0, 1, ps)
        store(0, 0)
        actC_half(5, tC[5], 1)
        ps = pe_h(2, 0)
        evac_h(2, 0, ps)
        load(3)
        store(0, 1)
        ps = pe_h(2, 1)
        evac_h(2, 1, ps)
        row_full_dve(7)
        load(4)
        store(1)
        load(6)
        tB[5] = ts_full(5, 2, bpool, "tB", "tB5")
        tA[5] = ts_full(5, 0, apool, "tA", "tA5")
        dve_add(tA[5], tC[5])             # s5
        dve_add(tA[5], tB[5])             # ot5
        ots[5] = tA[5]
        ps = pe_h(3, 0)
        evac_h(3, 0, ps)
        store(2, 0)
        ps = pe_h(3, 1)
        evac_h(3, 1, ps)
        store(2, 1)
        tC[6] = cpool.tile([128, L], bf16, tag="tC", name="tC6")
        nc.vector.tensor_scalar(
            tC[6][:, :], xps[6][:, 1 : 1 + L], wsl(6, 1), wsl(6, 3), MULT, ADD
        )
        tA[6] = ts_full(6, 0, apool, "tA", "tA6")
        tB[6] = ts_full(6, 2, bpool, "tB", "tB6")
        store(7)
        store(5)
        ps = pe_h(4, 0)
        evac_h(4, 0, ps)
        store(3, 0)
        dve_add(tA[6], tC[6])             # s6
        ps = pe_h(4, 1)
        evac_h(4, 1, ps)
        store(3, 1)
        dve_add(tA[6], tB[6])             # ot6
        ots[6] = tA[6]
        store(4, 0)
        store(4, 1)
        store(6)

    nc.compile()
    return nc


def _pack_weights(w, b):
    """[128, 4*NBLK] f32 with cols (w0, w1, w2, b) per channel block."""
    w = np.asarray(w, dtype=np.float32).reshape(C, K)
    b = np.asarray(b, dtype=np.float32)
    wp = np.zeros((128, 4 * NBLK), np.float32)
    for cb in range(NBLK):
        blk = slice(cb * 128, (cb + 1) * 128)
        wp[:, cb * 4 + 0] = w[blk, 0]
        wp[:, cb * 4 + 1] = w[blk, 1]
        wp[:, cb * 4 + 2] = w[blk, 2]
        wp[:, cb * 4 + 3] = b[blk]
    return wp


def _pack_diag(w):
    """[128, 3*NBLK*128] bf16: diag(w_k) per (block, tap)."""
    import ml_dtypes

    w = np.asarray(w, dtype=np.float32).reshape(C, K)
    wd = np.zeros((128, 3 * NBLK * 128), np.float32)
    for cb in range(NBLK):
        for k in range(3):
            o = (cb * 3 + k) * 128
            wd[np.arange(128), o + np.arange(128)] = w[cb * 128 : (cb + 1) * 128, k]
    return wd.astype(ml_dtypes.bfloat16)


def _run(inputs, trace=False, **kw):
    import ml_dtypes

    from concourse.bass_utils import run_bass_kernel_spmd

    if "nc" not in _STATE:
        _STATE["nc"] = _build_program()
    nc = _STATE["nc"]

    x = np.asarray(inputs["x"], dtype=np.float32)
    xq = np.zeros((B, C, LP), dtype=ml_dtypes.bfloat16)
    xq[:, :, 1 : L + 1] = x.astype(ml_dtypes.bfloat16)
    wp = _pack_weights(inputs["w"], inputs["b"])
    wd = _pack_diag(inputs["w"])
    in_maps = [
        {"x": xq[c * B_SH : (c + 1) * B_SH], "wpack": wp, "wdiag": wd}
        for c in range(N_CORES)
    ]
    res = run_bass_kernel_spmd(
        nc, in_maps, core_ids=list(range(N_CORES)), trace=trace, **kw
    )
    out = np.concatenate([res.results[c]["out"] for c in range(N_CORES)], axis=0)
    return out.astype(np.float32), res


def kernel(**inputs):
    return _run(inputs)[0]
